# revision 31
# baseline (speedup 1.0000x reference)
"""BitNet-style row-parallel linear on 8 TRN2 NeuronCores.

Reference computes: out[b,s,o] = sum_d x[b,s,d] * sign(w[o,d]) + bias[o]
  x: [4, 2048, 4096] f32, w: [4096, 4096] f32, bias: [4096] f32.

Strategy: data-parallel over the 8192 (b*s) rows — each of the 8 cores
computes a 1024-row slice of the output against the full binarized
weight. No collective needed; shards concatenate to the full output.
(The row-parallel/all-reduce hint costs a 128MB all-reduce per core;
sharding M instead makes the partial outputs disjoint.)

Precision: mixed split-K. The weights are exactly +-1 in ANY float
format, so the only quantization error is x's. The first KFP8=3584
reduction dims run as fp8e4 DoubleRow matmuls — 2 k-tiles (256 rows)
per instruction at 2x the bf16 PE rate — and the last 512 dims run in
bf16 to pull max-rel-err under the 2e-2 gate: measured 1.87e-2 on the
real inputs (vs 2.12e-2 all-fp8), deterministic (hw PSUM accumulation
matches the CPU model to ~6e-5). bf16 alone is 1.2e-3 but 1.5x slower;
fp8 hi+lo would be exact but cancels DoubleRow's speedup exactly.

Per (n-block, m-tile): 14 DR + 4 bf16 matmuls into one PSUM bank.
Block 0 runs k-outer/m-inner chasing the prologue DMAs; later blocks
m-outer/k-inner so evictions pipeline. W streams one n-block ahead,
issues spread across the consuming block so coalesced waits are
pre-satisfied. PE measured ~97% issue-rate efficiency at 2.4GHz
(216ns/512-row matmul); HW exec ~280us vs 437us bf16 PE roofline.
Beware: the part DVFS-throttles some runs to ~2.0GHz (+20% wall).
"""

import numpy as np

B, S, D_IN, D_OUT = 4, 2048, 4096, 4096
NCORES = 8
M_TOTAL = B * S
M_CORE = M_TOTAL // NCORES

import os

_cache = {}

# "f32r" (fp22 multiply, highest precision) or "bf16" (half the DMA
# traffic + fast weight load; weights are exactly representable).
DTYPE = os.environ.get("BK_DTYPE", "bf16")


IMPL = os.environ.get("BK_IMPL", "mixed")

# Mixed-precision split-K: first KFP8 of the 4096 reduction dims run as
# fp8e4 DoubleRow matmuls (2 k-tiles per instruction, 2x PE rate), the
# remaining dims in bf16. Weights are exactly +-1 in fp8; only x's
# rounding (rms 2.7e-2 per element) contributes error. Measured on the
# real inputs (hw matches the CPU fp8 sim bit-for-bit at 4 sig figs):
# KFP8=3072 -> max rel 1.702e-2, KFP8=3584 -> 1.886e-2 (gate 2e-2).
KFP8 = int(os.environ.get("BK_KFP8", "3584"))
# hi/lo mode: all-K fp8, x sent as (hi, lo) fp8 pair -> full precision,
# worth it only if hw DoubleRow is 4x (cost model) not 2x (public specs).
HILO = bool(int(os.environ.get("BK_HILO", "0")))


def _custom_body(nc, tc, kxm, kxn, out, mm_dt, mybir):
    """x^T stays SBUF-resident; sign(w)^T streams through once.

    Per n-block of 512 output columns, accumulate k into PSUM banks.
    Block 0 sweeps all 8 banks per k-tile (x still streaming in);
    later blocks run one bank at a time so evictions pipeline and the
    tail after the last matmul is a single evict+store.
    """
    P = 128
    KT = D_IN // P          # 32 k tiles
    MT = M_CORE // P        # 8 m tiles
    NW = 512
    NB = D_OUT // NW        # 8 n blocks
    f32 = mybir.dt.float32

    from contextlib import ExitStack
    with ExitStack() as ctx:
        kxm_pool = ctx.enter_context(tc.tile_pool(name="kxm", bufs=1))
        kxn_pool = ctx.enter_context(tc.tile_pool(name="kxn", bufs=9))
        psum_pool = ctx.enter_context(
            tc.tile_pool(name="psum", bufs=8, space="PSUM"))
        out_pool = ctx.enter_context(tc.tile_pool(name="outp", bufs=8))

        def issue_chunk(nb, c, k0, sz):
            # one kxn chunk: k tiles [k0, k0+sz) of n block nb
            t = kxn_pool.tile([P, sz, NW], mm_dt, tag="kxn",
                              name=f"kxn_{nb}_{c}", bufs=24)
            src = kxn[k0 * P:(k0 + sz) * P, nb * NW:(nb + 1) * NW]
            nc.sync.dma_start(
                out=t, in_=src.rearrange("(ko ki) n -> ki ko n", ki=P))
            return [t[:, i, :] for i in range(sz)]

        def issue_chunks(nb, sizes):
            rhs, k0 = [], 0
            for c, sz in enumerate(sizes):
                rhs += issue_chunk(nb, c, k0, sz)
                k0 += sz
            return rhs

        kxm_tiles = {}

        def issue_kxm(k, h):
            kt = kxm_pool.tile([P, M_CORE // 2], mm_dt, tag="kxm",
                               name=f"kxm_{k}_{h}", bufs=2 * KT)
            eng = nc.scalar if h == 0 else nc.gpsimd
            eng.dma_start(out=kt[:, :],
                          in_=kxm[k * P:(k + 1) * P,
                                  h * (M_CORE // 2):(h + 1) * (M_CORE // 2)])
            kxm_tiles[(k, h)] = kt

        def lhsT(k, m):
            h, off = divmod(m, MT // 2)
            return kxm_tiles[(k, h)][:, off * P:(off + 1) * P]

        # Prologue interleave: x low-halves arrive at sweep-A pace on
        # the scalar queue, weight chunks on sync; x high-halves (for
        # sweep B) trail on the gpsimd queue.
        sizes0 = [2, 2, 2, 2, 4, 4, 4, 4, 4, 4]
        rhs0, k0 = [], 0
        issue_kxm(0, 0)
        issue_kxm(1, 0)
        for c, sz in enumerate(sizes0):
            rhs0 += issue_chunk(0, c, k0, sz)
            k0 += sz
            for k in range(min(k0 + 2, KT)):
                if (k, 0) not in kxm_tiles:
                    issue_kxm(k, 0)
            for k in range(min(k0 - 8, KT)):
                if (k, 1) not in kxm_tiles:
                    issue_kxm(k, 1)
        for k in range(KT):
            if (k, 0) not in kxm_tiles:
                issue_kxm(k, 0)
        for k in range(KT):
            if (k, 1) not in kxm_tiles:
                issue_kxm(k, 1)

        next_rhs = rhs0
        for nb in range(NB):
            ncols = slice(nb * NW, (nb + 1) * NW)
            rhs_k = next_rhs
            psums = [psum_pool.tile([P, NW], f32, tag="ps", name=f"ps_{nb}_{i}")
                     for i in range(MT)]
            # Block 0: two 4-bank sweeps matched to the x-half arrival
            # rate; later blocks: one bank at a time (x resident).
            groups = [range(MT // 2), range(MT // 2, MT)] if nb == 0 \
                else [[m] for m in range(MT)]
            for gi, ms in enumerate(groups):
                for k in range(KT):
                    for m in ms:
                        nc.tensor.matmul(
                            psums[m][:, :],
                            lhsT=lhsT(k, m),
                            rhs=rhs_k[k],
                            start=(k == 0), stop=(k == KT - 1))
                if gi == 0 and nb + 1 < NB:
                    next_rhs = issue_chunks(nb + 1, [4] * 8)
                for m in ms:
                    ot = out_pool.tile([P, NW], f32, tag="ot", name=f"ot_{nb}_{m}")
                    nc.vector.tensor_copy(out=ot[:, :], in_=psums[m][:, :])
                    nc.gpsimd.dma_start(
                        out=out[m * P:(m + 1) * P, ncols], in_=ot[:, :])


def _warmup(nc, tc, mybir):
    # Warmup matmuls to pre-ramp the HAM-throttled PE clock turned out
    # to be a net LOSS: every instruction ahead of the first real matmul
    # inflates its coalesced DMA wait (12 warmups -> first real mm at
    # ~20.5us with a k=4 half-clock dip after the idle gap, 280.4us
    # total; 40 warmups or 12 fp32 ones: worse still). With ZERO
    # warmups the first real matmul issues at ~11.5us and the clock
    # ramps on real work: 275.6us measured. Keep BK_WARM=0.
    from contextlib import ExitStack
    with ExitStack() as ctx:
        wp = ctx.enter_context(tc.tile_pool(name="warm", bufs=1))
        wpp = ctx.enter_context(
            tc.tile_pool(name="warmp", bufs=1, space="PSUM"))
        wdt = (mybir.dt.float32 if os.environ.get("BK_WARM_F32", "0") == "1"
               else mybir.dt.bfloat16)
        a = wp.tile([128, 128], wdt)
        b = wp.tile([128, 512], wdt)
        nc.any.memset(a[:, :], 0.0)
        nc.any.memset(b[:, :], 0.0)
        ps = wpp.tile([128, 512], mybir.dt.float32)
        for _ in range(int(os.environ.get("BK_WARM", "0"))):
            nc.tensor.matmul(ps[:, :], lhsT=a[:, :], rhs=b[:, :],
                             start=True, stop=True)


def _mixed_body(nc, tc, xf, xb, wf, wb, out, mybir, KF, hilo):
    """Split-K mixed fp8/bf16 GEMM, x^T SBUF-resident, W streamed.

    fp8 segment: DoubleRow matmuls, lhsT [128, 2, 128] / rhs [128, 2, 512]
    cover 256 reduction rows per instruction. bf16 segment: classic
    [128,128]@[128,512]. Both accumulate into the same PSUM bank per
    (n-block, m-tile). Block 0 runs k-outer/m-inner so compute chases the
    prologue DMAs; later blocks run m-outer/k-inner so evictions pipeline.
    In hilo mode x rows are [hi;lo] fp8 and W tiles are reused for both.
    """
    P = 128
    NW = 512
    NB = D_OUT // NW            # 8 n blocks
    MT = M_CORE // P            # 8 m tiles / psum banks
    KFP = (2 * KF if hilo else KF) // 256   # fp8 pair-instructions per (nb,m)
    WFP = KF // 256                         # distinct fp8 W pair-tiles per nb
    KB = (D_IN - KF) // P                   # bf16 k tiles
    f32 = mybir.dt.float32
    f8 = mybir.dt.float8e4
    bf = mybir.dt.bfloat16
    DR = mybir.MatmulPerfMode.DoubleRow

    # Queue plan: sync's hw ring wakes first (~9us) — it carries the
    # block-0 prologue (split across sub-queue transfers so early tiles
    # don't serialize on one 23GB/s hw engine) plus the out stream;
    # scalar's ring carries the steady-state W stream for nb>=1.
    # Few big tiles + chunked subtile DMAs (instead of one tile per
    # chunk) keep the semaphore count low — engine boot and the final
    # clear_and_free storm both scale with it.
    from contextlib import ExitStack
    with ExitStack() as ctx:
        xpool = ctx.enter_context(tc.tile_pool(name="xp", bufs=1))
        wpool = ctx.enter_context(tc.tile_pool(name="wp", bufs=1))
        psum_pool = ctx.enter_context(
            tc.tile_pool(name="psum", bufs=8, space="PSUM"))
        out_pool = ctx.enter_context(tc.tile_pool(name="outp", bufs=1))

        xf_t = xpool.tile([P, 2 * KFP, M_CORE], f8, tag="xf", name="xf_sb",
                          bufs=1)
        xb_t = (xpool.tile([P, KB, M_CORE], bf, tag="xb", name="xb_sb",
                           bufs=1) if KB else None)

        def issue_xf(t, eng, ways=1):
            src = xf[t * P:(t + 1) * P, :].rearrange("p (i m) -> p i m", i=2)
            mw = M_CORE // ways
            for q in range(ways):
                ms = slice(q * mw, (q + 1) * mw)
                eng.dma_start(out=xf_t[:, 2 * t:2 * t + 2, ms],
                              in_=src[:, :, ms])

        def issue_xb(k, eng):
            eng.dma_start(out=xb_t[:, k, :], in_=xb[k * P:(k + 1) * P, :])

        def issue_wf(nb, wtile, t, eng, ways=1):
            j = nb * WFP + t
            src = wf[j * P:(j + 1) * P, :].rearrange("p (i n) -> p i n", i=2)
            nw = NW // ways
            for q in range(ways):
                ns = slice(q * nw, (q + 1) * nw)
                eng.dma_start(out=wtile[:, 2 * t:2 * t + 2, ns],
                              in_=src[:, :, ns])

        def issue_wb(nb, wtile, k, eng):
            eng.dma_start(out=wtile[:, k, :],
                          in_=wb[k * P:(k + 1) * P, nb * NW:(nb + 1) * NW])

        def alloc_w(nb):
            wft = wpool.tile([P, 2 * WFP, NW], f8, tag="wf",
                             name=f"wf_{nb}", bufs=2)
            wbt = (wpool.tile([P, KB, NW], bf, tag="wb", name=f"wb_{nb}",
                              bufs=2) if KB else None)
            return wft, wbt

        def issue_w(nb, eng):
            wft, wbt = alloc_w(nb)
            for t in range(WFP):
                issue_wf(nb, wft, t, eng)
            for k in range(KB):
                issue_wb(nb, wbt, k, eng)
            return wft, wbt

        def issue_w_chunks(nb, eng):
            """Chunked prefetch: yields after a couple of DMAs so issues
            spread across the consuming block's passes — a pass-head's
            coalesced wait then only covers long-landed transfers."""
            wft, wbt = alloc_w(nb)
            n = 0
            for t in range(WFP):
                issue_wf(nb, wft, t, eng)
                n += 1
                if n % 3 == 0:
                    yield wft, wbt
            for k in range(KB):
                issue_wb(nb, wbt, k, eng)
            yield wft, wbt

        # Prologue: first sweeps split 4-ways across hw sub-queues so
        # they land with low latency; sync and scalar alternate so both
        # rings pull their share of block 0.
        # NOTE: don't split finer than this — the scheduler's coalesced
        # first-wait scales with DMA *issue count* (8-way splitting made
        # the first matmul wait on >=128 issues, +14us).
        wft0, wbt0 = alloc_w(0)
        for t in range(WFP):
            ways = 4 if t < 2 else (2 if t < 4 else 1)
            eng = nc.sync if (t % 2 == 0 or t < 2) else nc.scalar
            issue_xf(t, eng, ways)
            issue_wf(0, wft0, t, nc.sync if (t % 2 == 0 or t < 2)
                     else nc.scalar, min(ways, 2))
        if hilo:
            for t in range(WFP, KFP):
                issue_xf(t, nc.sync if t % 2 == 0 else nc.scalar)
        for k in range(KB):
            issue_xb(k, nc.sync if k % 2 == 0 else nc.scalar)
            issue_wb(0, wbt0, k, nc.sync if k % 2 == 1 else nc.scalar)
        w_tiles = (wft0, wbt0)

        def mm_f8(bank, t, m, start, stop, wft):
            nc.tensor.matmul(
                bank[:, :],
                lhsT=xf_t[:, 2 * t:2 * t + 2, m * P:(m + 1) * P],
                rhs=wft[:, 2 * (t % WFP):2 * (t % WFP) + 2, :],
                start=start, stop=stop, perf_mode=DR)

        def mm_bf(bank, k, m, start, stop, wbt):
            nc.tensor.matmul(
                bank[:, :],
                lhsT=xb_t[:, k, m * P:(m + 1) * P],
                rhs=wbt[:, k, :],
                start=start, stop=stop)

        psums = [psum_pool.tile([P, NW], f32, tag="ps", name=f"ps_{m}")
                 for m in range(MT)]
        ots = [out_pool.tile([P, NW], f32, tag="ot", name=f"ot_{m}", bufs=8)
               for m in range(MT)]

        def evict(nb, m, bank):
            nc.vector.tensor_copy(out=ots[m][:, :], in_=bank[:, :])
            nc.sync.dma_start(
                out=out[m * P:(m + 1) * P, nb * NW:(nb + 1) * NW],
                in_=ots[m][:, :])

        for nb in range(NB):
            wft, wbt = w_tiles
            pf = (issue_w_chunks(nb + 1, nc.scalar) if nb + 1 < NB
                  else iter(()))
            if nb == 0:
                # k-outer, m-inner: each k tile feeds all 8 banks while
                # later tiles are still in flight.
                for t in range(KFP):
                    for m in range(MT):
                        mm_f8(psums[m], t, m, t == 0,
                              KB == 0 and t == KFP - 1, wft)
                    if t >= 1:
                        w_tiles = next(pf, w_tiles)
                for k in range(KB):
                    for m in range(MT):
                        mm_bf(psums[m], k, m, KFP == 0 and k == 0,
                              k == KB - 1, wbt)
                    w_tiles = next(pf, w_tiles)
                for m in range(MT):
                    evict(nb, m, psums[m])
            else:
                # All DR passes first, then all bf16 passes: the PE pays
                # the DoubleRow mode-entry (+187ns) once per block, not
                # once per bank pass (measured: BF->DR matmul 566ns vs
                # 379ns steady; DR->BF only +10ns).
                for m in range(MT):
                    for t in range(KFP):
                        mm_f8(psums[m], t, m, t == 0,
                              KB == 0 and t == KFP - 1, wft)
                    w_tiles = next(pf, w_tiles)
                    if KB == 0:
                        evict(nb, m, psums[m])
                for m in range(MT):
                    for k in range(KB):
                        mm_bf(psums[m], k, m, KFP == 0 and k == 0,
                              k == KB - 1, wbt)
                    if KB:
                        evict(nb, m, psums[m])
            for w_tiles in pf:
                pass
            w_tiles = w_tiles if nb + 1 < NB else None


def _build():
    """Build + compile the 8-core SPMD Bass program once per process."""
    if "nc" in _cache:
        return _cache["nc"]

    import concourse.bacc as bacc
    import concourse.tile as tile
    import concourse.mybir as mybir
    from concourse.kernels.tile_matmul import matmul_tile_kernel

    mm_dt = {"f32r": mybir.dt.float32r, "bf16": mybir.dt.bfloat16}[DTYPE]

    nc = bacc.Bacc("TRN2", target_bir_lowering=False, debug=False,
                   enable_asserts=bool(os.environ.get("BK_ASSERTS")),
                   num_devices=int(os.environ.get("BK_NDEV", NCORES)))
    if IMPL == "mixed":
        KF, KBR = KFP8, D_IN - KFP8
        # xf rows are pre-tiled [t, p] with 2048B (i,m) lines; wf rows
        # are [nb, t, p] with 1024B (i,n) lines — single-burst DMAs.
        xf = nc.dram_tensor("xf", [(2 * KF if HILO else KF) // 2, 2 * M_CORE],
                            mybir.dt.float8e4, kind="ExternalInput").ap()
        wf = nc.dram_tensor("wf", [(D_OUT // 512) * (KF // 2), 1024],
                            mybir.dt.float8e4, kind="ExternalInput").ap()
        xb = wb = None
        if KBR:
            xb = nc.dram_tensor("xb", [KBR, M_CORE], mybir.dt.bfloat16,
                                kind="ExternalInput").ap()
            wb = nc.dram_tensor("wb", [KBR, D_OUT], mybir.dt.bfloat16,
                                kind="ExternalInput").ap()
        out = nc.dram_tensor("out", [M_CORE, D_OUT], mybir.dt.float32,
                             kind="ExternalOutput").ap()

        with tile.TileContext(nc) as tc:
            _warmup(nc, tc, mybir)
            _mixed_body(nc, tc, xf, xb, wf, wb, out, mybir, KFP8, HILO)
        nc.compile()
        _cache["nc"] = nc
        return nc
    kxm = nc.dram_tensor("kxm", [D_IN, M_CORE], mm_dt,
                         kind="ExternalInput").ap()
    kxn = nc.dram_tensor("kxn", [D_IN, D_OUT], mm_dt,
                         kind="ExternalInput").ap()
    out = nc.dram_tensor("out", [M_CORE, D_OUT], mybir.dt.float32,
                         kind="ExternalOutput").ap()
    if IMPL == "custom":
        with tile.TileContext(nc) as tc:
            _warmup(nc, tc, mybir)
            _custom_body(nc, tc, kxm, kxn, out, mm_dt, mybir)
    else:
        kw = {}
        if os.environ.get("BK_MAX_K_TILE"):
            kw["MAX_K_TILE_SIZE"] = int(os.environ["BK_MAX_K_TILE"])
        if os.environ.get("BK_SKIP_K_SNAKE"):
            kw["skip_k_snake"] = True
        if os.environ.get("BK_NO_CACHE_TILES"):
            kw["cache_tiles"] = False
        with tile.TileContext(nc) as tc:
            _warmup(nc, tc, mybir)
            matmul_tile_kernel(tc, kxm, kxn, out, **kw)
    nc.compile()
    _cache["nc"] = nc
    return nc


def _prep_inputs_mixed(x, weight):
    import ml_dtypes
    f8 = ml_dtypes.float8_e4m3
    bf = ml_dtypes.bfloat16
    KF = KFP8
    x2d = np.asarray(x, dtype=np.float32).reshape(M_TOTAL, D_IN)
    st = np.sign(weight, dtype=np.float32).T  # [D_IN, D_OUT]
    # wf host layout [nb, t, p, i, n] -> each DMA line contiguous
    wf = np.ascontiguousarray(
        st[:KF].astype(f8).reshape(KF // 256, 2, 128, D_OUT // 512, 512)
        .transpose(3, 0, 2, 1, 4).reshape(-1, 1024))
    wb = (np.ascontiguousarray(st[KF:].astype(bf))
          if KF < D_IN else None)
    in_maps = []
    for c in range(NCORES):
        xT = np.ascontiguousarray(x2d[c * M_CORE:(c + 1) * M_CORE].T)
        if HILO:
            hi = xT[:KF].astype(f8)
            lo = (xT[:KF] - hi.astype(np.float32)).astype(f8)
            xfc = np.concatenate([hi, lo], axis=0)
        else:
            xfc = xT[:KF].astype(f8)
        # xf host layout [t, p, i, m]
        xfc = (xfc.reshape(-1, 2, 128, M_CORE).transpose(0, 2, 1, 3)
               .reshape(-1, 2 * M_CORE))
        m = {"xf": np.ascontiguousarray(xfc), "wf": wf}
        if wb is not None:
            m["xb"] = np.ascontiguousarray(xT[KF:].astype(bf))
            m["wb"] = wb
        in_maps.append(m)
    return in_maps


def _prep_inputs(x, weight):
    if DTYPE == "bf16":
        import ml_dtypes
        np_dt = ml_dtypes.bfloat16
    else:
        np_dt = np.float32
    x2d = np.asarray(x, dtype=np.float32).reshape(M_TOTAL, D_IN)
    kxn = np.ascontiguousarray(np.sign(weight, dtype=np.float32).T.astype(np_dt))
    in_maps = []
    for c in range(NCORES):
        kxm = np.ascontiguousarray(x2d[c * M_CORE:(c + 1) * M_CORE].T.astype(np_dt))
        in_maps.append({"kxm": kxm, "kxn": kxn})
    return in_maps


def _run(x, weight, bias, trace=False):
    from concourse.bass_utils import run_bass_kernel_spmd

    nc = _build()
    in_maps = (_prep_inputs_mixed(x, weight) if IMPL == "mixed"
               else _prep_inputs(x, weight))
    res = run_bass_kernel_spmd(nc, in_maps, core_ids=list(range(NCORES)),
                               trace=trace)
    out = np.concatenate([res.results[c]["out"] for c in range(NCORES)],
                         axis=0)
    bias = np.asarray(bias, dtype=np.float32)
    if np.any(bias):
        out += bias
    return out.reshape(B, S, D_OUT), res


def kernel(x, weight, bias):
    out, _ = _run(x, weight, bias, trace=False)
    return out



# revision 32
# speedup vs baseline: 1.0297x; 1.0297x over previous
"""BitNet-style row-parallel linear on 8 TRN2 NeuronCores.

Reference computes: out[b,s,o] = sum_d x[b,s,d] * sign(w[o,d]) + bias[o]
  x: [4, 2048, 4096] f32, w: [4096, 4096] f32, bias: [4096] f32.

Strategy: data-parallel over the 8192 (b*s) rows — each of the 8 cores
computes a 1024-row slice of the output against the full binarized
weight. No collective needed; shards concatenate to the full output.
(The row-parallel/all-reduce hint costs a 128MB all-reduce per core;
sharding M instead makes the partial outputs disjoint.)

Precision: mixed split-K. The weights are exactly +-1 in ANY float
format, so the only quantization error is x's. The first KFP8=3584
reduction dims run as fp8e4 DoubleRow matmuls — 2 k-tiles (256 rows)
per instruction at 2x the bf16 PE rate — and the last 512 dims run in
bf16 to pull max-rel-err under the 2e-2 gate: measured 1.87e-2 on the
real inputs (vs 2.12e-2 all-fp8), deterministic (hw PSUM accumulation
matches the CPU model to ~6e-5). bf16 alone is 1.2e-3 but 1.5x slower;
fp8 hi+lo would be exact but cancels DoubleRow's speedup exactly.

Per (n-block, m-tile): 14 DR + 4 bf16 matmuls into one PSUM bank.
Block 0 runs k-outer/m-inner chasing the prologue DMAs; later blocks
m-outer/k-inner so evictions pipeline. W streams one n-block ahead,
issues spread across the consuming block so coalesced waits are
pre-satisfied. PE measured ~97% issue-rate efficiency at 2.4GHz
(216ns/512-row matmul); HW exec ~280us vs 437us bf16 PE roofline.
Beware: the part DVFS-throttles some runs to ~2.0GHz (+20% wall).
"""

import numpy as np

B, S, D_IN, D_OUT = 4, 2048, 4096, 4096
NCORES = 8
M_TOTAL = B * S
M_CORE = M_TOTAL // NCORES

import os

_cache = {}

# "f32r" (fp22 multiply, highest precision) or "bf16" (half the DMA
# traffic + fast weight load; weights are exactly representable).
DTYPE = os.environ.get("BK_DTYPE", "bf16")


IMPL = os.environ.get("BK_IMPL", "mixed")

# Mixed-precision split-K: first KFP8 of the 4096 reduction dims run as
# fp8e4 DoubleRow matmuls (2 k-tiles per instruction, 2x PE rate), the
# remaining dims in bf16. Weights are exactly +-1 in fp8; only x's
# rounding (rms 2.7e-2 per element) contributes error. Measured on the
# real inputs (hw matches the CPU fp8 sim bit-for-bit at 4 sig figs):
# KFP8=3072 -> max rel 1.702e-2, KFP8=3584 -> 1.886e-2 (gate 2e-2).
KFP8 = int(os.environ.get("BK_KFP8", "3584"))
# hi/lo mode: all-K fp8, x sent as (hi, lo) fp8 pair -> full precision,
# worth it only if hw DoubleRow is 4x (cost model) not 2x (public specs).
HILO = bool(int(os.environ.get("BK_HILO", "0")))


def _custom_body(nc, tc, kxm, kxn, out, mm_dt, mybir):
    """x^T stays SBUF-resident; sign(w)^T streams through once.

    Per n-block of 512 output columns, accumulate k into PSUM banks.
    Block 0 sweeps all 8 banks per k-tile (x still streaming in);
    later blocks run one bank at a time so evictions pipeline and the
    tail after the last matmul is a single evict+store.
    """
    P = 128
    KT = D_IN // P          # 32 k tiles
    MT = M_CORE // P        # 8 m tiles
    NW = 512
    NB = D_OUT // NW        # 8 n blocks
    f32 = mybir.dt.float32

    from contextlib import ExitStack
    with ExitStack() as ctx:
        kxm_pool = ctx.enter_context(tc.tile_pool(name="kxm", bufs=1))
        kxn_pool = ctx.enter_context(tc.tile_pool(name="kxn", bufs=9))
        psum_pool = ctx.enter_context(
            tc.tile_pool(name="psum", bufs=8, space="PSUM"))
        out_pool = ctx.enter_context(tc.tile_pool(name="outp", bufs=8))

        def issue_chunk(nb, c, k0, sz):
            # one kxn chunk: k tiles [k0, k0+sz) of n block nb
            t = kxn_pool.tile([P, sz, NW], mm_dt, tag="kxn",
                              name=f"kxn_{nb}_{c}", bufs=24)
            src = kxn[k0 * P:(k0 + sz) * P, nb * NW:(nb + 1) * NW]
            nc.sync.dma_start(
                out=t, in_=src.rearrange("(ko ki) n -> ki ko n", ki=P))
            return [t[:, i, :] for i in range(sz)]

        def issue_chunks(nb, sizes):
            rhs, k0 = [], 0
            for c, sz in enumerate(sizes):
                rhs += issue_chunk(nb, c, k0, sz)
                k0 += sz
            return rhs

        kxm_tiles = {}

        def issue_kxm(k, h):
            kt = kxm_pool.tile([P, M_CORE // 2], mm_dt, tag="kxm",
                               name=f"kxm_{k}_{h}", bufs=2 * KT)
            eng = nc.scalar if h == 0 else nc.gpsimd
            eng.dma_start(out=kt[:, :],
                          in_=kxm[k * P:(k + 1) * P,
                                  h * (M_CORE // 2):(h + 1) * (M_CORE // 2)])
            kxm_tiles[(k, h)] = kt

        def lhsT(k, m):
            h, off = divmod(m, MT // 2)
            return kxm_tiles[(k, h)][:, off * P:(off + 1) * P]

        # Prologue interleave: x low-halves arrive at sweep-A pace on
        # the scalar queue, weight chunks on sync; x high-halves (for
        # sweep B) trail on the gpsimd queue.
        sizes0 = [2, 2, 2, 2, 4, 4, 4, 4, 4, 4]
        rhs0, k0 = [], 0
        issue_kxm(0, 0)
        issue_kxm(1, 0)
        for c, sz in enumerate(sizes0):
            rhs0 += issue_chunk(0, c, k0, sz)
            k0 += sz
            for k in range(min(k0 + 2, KT)):
                if (k, 0) not in kxm_tiles:
                    issue_kxm(k, 0)
            for k in range(min(k0 - 8, KT)):
                if (k, 1) not in kxm_tiles:
                    issue_kxm(k, 1)
        for k in range(KT):
            if (k, 0) not in kxm_tiles:
                issue_kxm(k, 0)
        for k in range(KT):
            if (k, 1) not in kxm_tiles:
                issue_kxm(k, 1)

        next_rhs = rhs0
        for nb in range(NB):
            ncols = slice(nb * NW, (nb + 1) * NW)
            rhs_k = next_rhs
            psums = [psum_pool.tile([P, NW], f32, tag="ps", name=f"ps_{nb}_{i}")
                     for i in range(MT)]
            # Block 0: two 4-bank sweeps matched to the x-half arrival
            # rate; later blocks: one bank at a time (x resident).
            groups = [range(MT // 2), range(MT // 2, MT)] if nb == 0 \
                else [[m] for m in range(MT)]
            for gi, ms in enumerate(groups):
                for k in range(KT):
                    for m in ms:
                        nc.tensor.matmul(
                            psums[m][:, :],
                            lhsT=lhsT(k, m),
                            rhs=rhs_k[k],
                            start=(k == 0), stop=(k == KT - 1))
                if gi == 0 and nb + 1 < NB:
                    next_rhs = issue_chunks(nb + 1, [4] * 8)
                for m in ms:
                    ot = out_pool.tile([P, NW], f32, tag="ot", name=f"ot_{nb}_{m}")
                    nc.vector.tensor_copy(out=ot[:, :], in_=psums[m][:, :])
                    nc.gpsimd.dma_start(
                        out=out[m * P:(m + 1) * P, ncols], in_=ot[:, :])


def _warmup(nc, tc, mybir):
    # Warmup matmuls to pre-ramp the HAM-throttled PE clock turned out
    # to be a net LOSS: every instruction ahead of the first real matmul
    # inflates its coalesced DMA wait (12 warmups -> first real mm at
    # ~20.5us with a k=4 half-clock dip after the idle gap, 280.4us
    # total; 40 warmups or 12 fp32 ones: worse still). With ZERO
    # warmups the first real matmul issues at ~11.5us and the clock
    # ramps on real work: 275.6us measured. Keep BK_WARM=0.
    from contextlib import ExitStack
    with ExitStack() as ctx:
        wp = ctx.enter_context(tc.tile_pool(name="warm", bufs=1))
        wpp = ctx.enter_context(
            tc.tile_pool(name="warmp", bufs=1, space="PSUM"))
        wdt = (mybir.dt.float32 if os.environ.get("BK_WARM_F32", "0") == "1"
               else mybir.dt.bfloat16)
        a = wp.tile([128, 128], wdt)
        b = wp.tile([128, 512], wdt)
        nc.any.memset(a[:, :], 0.0)
        nc.any.memset(b[:, :], 0.0)
        ps = wpp.tile([128, 512], mybir.dt.float32)
        for _ in range(int(os.environ.get("BK_WARM", "0"))):
            nc.tensor.matmul(ps[:, :], lhsT=a[:, :], rhs=b[:, :],
                             start=True, stop=True)


def _mixed_body(nc, tc, xf, xb, wf, wb, out, mybir, KF, hilo):
    """Split-K mixed fp8/bf16 GEMM, x^T SBUF-resident, W streamed.

    fp8 segment: DoubleRow matmuls, lhsT [128, 2, 128] / rhs [128, 2, 512]
    cover 256 reduction rows per instruction. bf16 segment: classic
    [128,128]@[128,512]. Both accumulate into the same PSUM bank per
    (n-block, m-tile). Block 0 runs k-outer/m-inner so compute chases the
    prologue DMAs; later blocks run m-outer/k-inner so evictions pipeline.
    In hilo mode x rows are [hi;lo] fp8 and W tiles are reused for both.
    """
    P = 128
    NW = 512
    NB = D_OUT // NW            # 8 n blocks
    MT = M_CORE // P            # 8 m tiles / psum banks
    KFP = (2 * KF if hilo else KF) // 256   # fp8 pair-instructions per (nb,m)
    WFP = KF // 256                         # distinct fp8 W pair-tiles per nb
    KB = (D_IN - KF) // P                   # bf16 k tiles
    f32 = mybir.dt.float32
    f8 = mybir.dt.float8e4
    bf = mybir.dt.bfloat16
    DR = mybir.MatmulPerfMode.DoubleRow

    # Queue plan: sync's hw ring wakes first (~9us) — it carries the
    # block-0 prologue (split across sub-queue transfers so early tiles
    # don't serialize on one 23GB/s hw engine) plus the out stream;
    # scalar's ring carries the steady-state W stream for nb>=1.
    # Few big tiles + chunked subtile DMAs (instead of one tile per
    # chunk) keep the semaphore count low — engine boot and the final
    # clear_and_free storm both scale with it.
    from contextlib import ExitStack
    with ExitStack() as ctx:
        xpool = ctx.enter_context(tc.tile_pool(name="xp", bufs=1))
        wpool = ctx.enter_context(tc.tile_pool(name="wp", bufs=1))
        psum_pool = ctx.enter_context(
            tc.tile_pool(name="psum", bufs=8, space="PSUM"))
        out_pool = ctx.enter_context(tc.tile_pool(name="outp", bufs=1))

        xf_t = xpool.tile([P, 2 * KFP, M_CORE], f8, tag="xf", name="xf_sb",
                          bufs=1)
        xb_t = (xpool.tile([P, KB, M_CORE], bf, tag="xb", name="xb_sb",
                           bufs=1) if KB else None)

        def issue_xf(t, eng, ways=1):
            src = xf[t * P:(t + 1) * P, :].rearrange("p (i m) -> p i m", i=2)
            mw = M_CORE // ways
            for q in range(ways):
                ms = slice(q * mw, (q + 1) * mw)
                eng.dma_start(out=xf_t[:, 2 * t:2 * t + 2, ms],
                              in_=src[:, :, ms])

        def issue_xb(k, eng):
            eng.dma_start(out=xb_t[:, k, :], in_=xb[k * P:(k + 1) * P, :])

        def issue_wf(nb, wtile, t, eng, ways=1):
            j = nb * WFP + t
            src = wf[j * P:(j + 1) * P, :].rearrange("p (i n) -> p i n", i=2)
            nw = NW // ways
            for q in range(ways):
                ns = slice(q * nw, (q + 1) * nw)
                eng.dma_start(out=wtile[:, 2 * t:2 * t + 2, ns],
                              in_=src[:, :, ns])

        def issue_wb(nb, wtile, k, eng):
            eng.dma_start(out=wtile[:, k, :],
                          in_=wb[k * P:(k + 1) * P, nb * NW:(nb + 1) * NW])

        def alloc_w(nb):
            wft = wpool.tile([P, 2 * WFP, NW], f8, tag="wf",
                             name=f"wf_{nb}", bufs=2)
            wbt = (wpool.tile([P, KB, NW], bf, tag="wb", name=f"wb_{nb}",
                              bufs=2) if KB else None)
            return wft, wbt

        def issue_w(nb, eng):
            wft, wbt = alloc_w(nb)
            for t in range(WFP):
                issue_wf(nb, wft, t, eng)
            for k in range(KB):
                issue_wb(nb, wbt, k, eng)
            return wft, wbt

        def issue_w_chunks(nb, eng):
            """Chunked prefetch: yields after a couple of DMAs so issues
            spread across the consuming block's passes — a pass-head's
            coalesced wait then only covers long-landed transfers."""
            wft, wbt = alloc_w(nb)
            n = 0
            for t in range(WFP):
                issue_wf(nb, wft, t, eng)
                n += 1
                if n % 3 == 0:
                    yield wft, wbt
            for k in range(KB):
                issue_wb(nb, wbt, k, eng)
            yield wft, wbt

        # Prologue: first sweeps split 4-ways across hw sub-queues so
        # they land with low latency; sync and scalar alternate so both
        # rings pull their share of block 0.
        # NOTE: don't split finer than this — the scheduler's coalesced
        # first-wait scales with DMA *issue count* (8-way splitting made
        # the first matmul wait on >=128 issues, +14us).
        # Strict parity split: odd sweeps (incl. t=1) ride the scalar
        # ring, which wakes ~15-16us — right when sweep 1 is consumed.
        # Keeping t=1 on sync serialized it behind sweep 0 on the slow
        # early ring (4.7us PE gap at ~16us, delaying full clock to 24us).
        wft0, wbt0 = alloc_w(0)
        for t in range(WFP):
            ways = 4 if t < 2 else (2 if t < 4 else 1)
            eng = nc.sync if t % 2 == 0 else nc.scalar
            issue_xf(t, eng, ways)
            issue_wf(0, wft0, t, eng, min(ways, 2))
        if hilo:
            for t in range(WFP, KFP):
                issue_xf(t, nc.sync if t % 2 == 0 else nc.scalar)
        for k in range(KB):
            issue_xb(k, nc.sync if k % 2 == 0 else nc.scalar)
            issue_wb(0, wbt0, k, nc.sync if k % 2 == 1 else nc.scalar)
        w_tiles = (wft0, wbt0)

        def mm_f8(bank, t, m, start, stop, wft):
            nc.tensor.matmul(
                bank[:, :],
                lhsT=xf_t[:, 2 * t:2 * t + 2, m * P:(m + 1) * P],
                rhs=wft[:, 2 * (t % WFP):2 * (t % WFP) + 2, :],
                start=start, stop=stop, perf_mode=DR)

        def mm_bf(bank, k, m, start, stop, wbt):
            nc.tensor.matmul(
                bank[:, :],
                lhsT=xb_t[:, k, m * P:(m + 1) * P],
                rhs=wbt[:, k, :],
                start=start, stop=stop)

        psums = [psum_pool.tile([P, NW], f32, tag="ps", name=f"ps_{m}")
                 for m in range(MT)]
        ots = [out_pool.tile([P, NW], f32, tag="ot", name=f"ot_{m}", bufs=8)
               for m in range(MT)]

        def evict(nb, m, bank):
            nc.vector.tensor_copy(out=ots[m][:, :], in_=bank[:, :])
            nc.sync.dma_start(
                out=out[m * P:(m + 1) * P, nb * NW:(nb + 1) * NW],
                in_=ots[m][:, :])

        for nb in range(NB):
            wft, wbt = w_tiles
            pf = (issue_w_chunks(nb + 1, nc.scalar) if nb + 1 < NB
                  else iter(()))
            if nb == 0:
                # k-outer, m-inner: each k tile feeds all 8 banks while
                # later tiles are still in flight.
                for t in range(KFP):
                    for m in range(MT):
                        mm_f8(psums[m], t, m, t == 0,
                              KB == 0 and t == KFP - 1, wft)
                    if t >= 1:
                        w_tiles = next(pf, w_tiles)
                for k in range(KB):
                    for m in range(MT):
                        mm_bf(psums[m], k, m, KFP == 0 and k == 0,
                              k == KB - 1, wbt)
                    w_tiles = next(pf, w_tiles)
                for m in range(MT):
                    evict(nb, m, psums[m])
            else:
                # All DR passes first, then all bf16 passes: the PE pays
                # the DoubleRow mode-entry (+187ns) once per block, not
                # once per bank pass (measured: BF->DR matmul 566ns vs
                # 379ns steady; DR->BF only +10ns).
                for m in range(MT):
                    for t in range(KFP):
                        mm_f8(psums[m], t, m, t == 0,
                              KB == 0 and t == KFP - 1, wft)
                    w_tiles = next(pf, w_tiles)
                    if KB == 0:
                        evict(nb, m, psums[m])
                for m in range(MT):
                    for k in range(KB):
                        mm_bf(psums[m], k, m, KFP == 0 and k == 0,
                              k == KB - 1, wbt)
                    if KB:
                        evict(nb, m, psums[m])
            for w_tiles in pf:
                pass
            w_tiles = w_tiles if nb + 1 < NB else None


def _build():
    """Build + compile the 8-core SPMD Bass program once per process."""
    if "nc" in _cache:
        return _cache["nc"]

    import concourse.bacc as bacc
    import concourse.tile as tile
    import concourse.mybir as mybir
    from concourse.kernels.tile_matmul import matmul_tile_kernel

    mm_dt = {"f32r": mybir.dt.float32r, "bf16": mybir.dt.bfloat16}[DTYPE]

    nc = bacc.Bacc("TRN2", target_bir_lowering=False, debug=False,
                   enable_asserts=bool(os.environ.get("BK_ASSERTS")),
                   num_devices=int(os.environ.get("BK_NDEV", NCORES)))
    if IMPL == "mixed":
        KF, KBR = KFP8, D_IN - KFP8
        # xf rows are pre-tiled [t, p] with 2048B (i,m) lines; wf rows
        # are [nb, t, p] with 1024B (i,n) lines — single-burst DMAs.
        xf = nc.dram_tensor("xf", [(2 * KF if HILO else KF) // 2, 2 * M_CORE],
                            mybir.dt.float8e4, kind="ExternalInput").ap()
        wf = nc.dram_tensor("wf", [(D_OUT // 512) * (KF // 2), 1024],
                            mybir.dt.float8e4, kind="ExternalInput").ap()
        xb = wb = None
        if KBR:
            xb = nc.dram_tensor("xb", [KBR, M_CORE], mybir.dt.bfloat16,
                                kind="ExternalInput").ap()
            wb = nc.dram_tensor("wb", [KBR, D_OUT], mybir.dt.bfloat16,
                                kind="ExternalInput").ap()
        out = nc.dram_tensor("out", [M_CORE, D_OUT], mybir.dt.float32,
                             kind="ExternalOutput").ap()

        with tile.TileContext(nc) as tc:
            _warmup(nc, tc, mybir)
            _mixed_body(nc, tc, xf, xb, wf, wb, out, mybir, KFP8, HILO)
        nc.compile()
        _cache["nc"] = nc
        return nc
    kxm = nc.dram_tensor("kxm", [D_IN, M_CORE], mm_dt,
                         kind="ExternalInput").ap()
    kxn = nc.dram_tensor("kxn", [D_IN, D_OUT], mm_dt,
                         kind="ExternalInput").ap()
    out = nc.dram_tensor("out", [M_CORE, D_OUT], mybir.dt.float32,
                         kind="ExternalOutput").ap()
    if IMPL == "custom":
        with tile.TileContext(nc) as tc:
            _warmup(nc, tc, mybir)
            _custom_body(nc, tc, kxm, kxn, out, mm_dt, mybir)
    else:
        kw = {}
        if os.environ.get("BK_MAX_K_TILE"):
            kw["MAX_K_TILE_SIZE"] = int(os.environ["BK_MAX_K_TILE"])
        if os.environ.get("BK_SKIP_K_SNAKE"):
            kw["skip_k_snake"] = True
        if os.environ.get("BK_NO_CACHE_TILES"):
            kw["cache_tiles"] = False
        with tile.TileContext(nc) as tc:
            _warmup(nc, tc, mybir)
            matmul_tile_kernel(tc, kxm, kxn, out, **kw)
    nc.compile()
    _cache["nc"] = nc
    return nc


def _prep_inputs_mixed(x, weight):
    import ml_dtypes
    f8 = ml_dtypes.float8_e4m3
    bf = ml_dtypes.bfloat16
    KF = KFP8
    x2d = np.asarray(x, dtype=np.float32).reshape(M_TOTAL, D_IN)
    st = np.sign(weight, dtype=np.float32).T  # [D_IN, D_OUT]
    # wf host layout [nb, t, p, i, n] -> each DMA line contiguous
    wf = np.ascontiguousarray(
        st[:KF].astype(f8).reshape(KF // 256, 2, 128, D_OUT // 512, 512)
        .transpose(3, 0, 2, 1, 4).reshape(-1, 1024))
    wb = (np.ascontiguousarray(st[KF:].astype(bf))
          if KF < D_IN else None)
    in_maps = []
    for c in range(NCORES):
        xT = np.ascontiguousarray(x2d[c * M_CORE:(c + 1) * M_CORE].T)
        if HILO:
            hi = xT[:KF].astype(f8)
            lo = (xT[:KF] - hi.astype(np.float32)).astype(f8)
            xfc = np.concatenate([hi, lo], axis=0)
        else:
            xfc = xT[:KF].astype(f8)
        # xf host layout [t, p, i, m]
        xfc = (xfc.reshape(-1, 2, 128, M_CORE).transpose(0, 2, 1, 3)
               .reshape(-1, 2 * M_CORE))
        m = {"xf": np.ascontiguousarray(xfc), "wf": wf}
        if wb is not None:
            m["xb"] = np.ascontiguousarray(xT[KF:].astype(bf))
            m["wb"] = wb
        in_maps.append(m)
    return in_maps


def _prep_inputs(x, weight):
    if DTYPE == "bf16":
        import ml_dtypes
        np_dt = ml_dtypes.bfloat16
    else:
        np_dt = np.float32
    x2d = np.asarray(x, dtype=np.float32).reshape(M_TOTAL, D_IN)
    kxn = np.ascontiguousarray(np.sign(weight, dtype=np.float32).T.astype(np_dt))
    in_maps = []
    for c in range(NCORES):
        kxm = np.ascontiguousarray(x2d[c * M_CORE:(c + 1) * M_CORE].T.astype(np_dt))
        in_maps.append({"kxm": kxm, "kxn": kxn})
    return in_maps


def _run(x, weight, bias, trace=False):
    from concourse.bass_utils import run_bass_kernel_spmd

    nc = _build()
    in_maps = (_prep_inputs_mixed(x, weight) if IMPL == "mixed"
               else _prep_inputs(x, weight))
    res = run_bass_kernel_spmd(nc, in_maps, core_ids=list(range(NCORES)),
                               trace=trace)
    out = np.concatenate([res.results[c]["out"] for c in range(NCORES)],
                         axis=0)
    bias = np.asarray(bias, dtype=np.float32)
    if np.any(bias):
        out += bias
    return out.reshape(B, S, D_OUT), res


def kernel(x, weight, bias):
    out, _ = _run(x, weight, bias, trace=False)
    return out

